# revision 1
# baseline (speedup 1.0000x reference)
"""Trainium2 Bass kernel for a GNN message-passing layer (GCL).

Reference computation:
    src = features[rows]; dst = features[cols]
    h = sigmoid(concat(src, dst) @ Wm1 + bm1)
    messages = softsign(h @ Wm2 + bm2)
    agg = segment_sum(messages, rows, N)
    g = sigmoid(concat(features, agg, time_embedding))
    g = sigmoid(g @ Wf1 + bf1)
    out = softsign(g @ Wf2 + bf2)

Restructure: concat(src, dst) @ Wm1 = A[rows] + B[cols] with A = X@Wm1[:F],
B = X@Wm1[F:] precomputed per node. All per-edge row movement is expressed
as one-hot expand matmuls on the tensor engine (256B-row DMA gathers are
descriptor-bound and infeasible at 650k rows):

  - edges are bucketed by (row-window w of 128 nodes, col-chunk c of CH
    nodes); one 128-edge tile per bucket, CH chosen so no bucket overflows;
  - S^T[m1, e] = A_w-expand + B_c-expand accumulated in PSUM via one-hot
    rhs matrices PR[n, e] and PC[p, e];
  - h^T = sigmoid(S^T + bm1)  (bias rides the partition dim for free);
  - msgs[e, m2] = (h^T-slice as lhsT) @ Wm2;  y = softsign(msgs);
  - aggT[m2, n] += (y as lhsT) @ P[e, n]  (the segment-sum).

Sharding: core k owns node range [k*1280, (k+1)*1280) (nodes padded to
10240) and all edges whose row lands there; each core computes the full B
table locally so no collectives are needed; outputs are concatenated on the
host. The per-core program is identical (SPMD); all per-core structure lives
in the input data (one-hot matrices, own-range feature slices).
"""

import numpy as np
import ml_dtypes

import concourse.bass as bass
import concourse.bacc as bacc
import concourse.mybir as mybir
import concourse.tile as tile
import concourse.dve_ops as dve_ops
from concourse.bass_utils import run_bass_kernel_spmd
from concourse.dve_ops import DveOp, RECIP_APPROX_FAST_CONSTS
from concourse.dve_spec import AluOp as DAlu, Bin, C0, C1, One, Spec, Src0, \
    Src1, lower
from concourse.dve_uop import DveOpSpec
from concourse.masks import make_identity
from concourse.mybir import ActivationFunctionType as AF, AluOpType as ALU

BF16 = mybir.dt.bfloat16
F32 = mybir.dt.float32
FP8 = mybir.dt.float8e4
NPBF16 = ml_dtypes.bfloat16
NPFP8 = ml_dtypes.float8_e4m3

N = 10000
E = 640000
FD = 128
NCORES = 8
NPAD = 10240
NT = NPAD // 128         # 80 row windows total
NTC = NT // NCORES       # 10 windows per core
RANGE = NPAD // NCORES   # 1280 nodes per core
GROUP = 4                # tiles per elementwise batch (free dim 512)

# one-hot matrix dtype: fp8e4 (0/1 exact, halves DMA vs bf16)
ONEHOT_DT = FP8
ONEHOT_NP = NPFP8


def _register_softsign_op():
    """Fused softsign tail as one custom DVE op:
        out = Src1 * y1,  d = Src0 + 1,  y0 = bitcast(~d)*c0,
        y1 = y0*(c1 - d*y0)
    i.e. out ~= Src1 / (1 + Src0) at ~0.5% rel (seed + one Newton pass).
    With Src0=|m|, Src1=m this is softsign(m) in a single DVE pass."""
    name = "INC1_RECIP_MUL_ANT"
    for existing in dve_ops.OPS:
        if existing.name == name:
            return existing

    def _ref(in0, in1, c0, c1, c2):
        d = in0.astype(np.float32) + 1.0
        not_d = (~d.view(np.int32)).view(np.float32)
        y0 = not_d * c0
        return in1.astype(np.float32) * (y0 * (c1 - d * y0))

    d = Bin(DAlu.ADD, Src0, One)
    not_d = Bin(DAlu.BITWISE_NOT, d, d)
    y0 = not_d * C0
    y1 = y0 * (C1 - d * y0)
    spec = Spec(body=Src1 * y1, reference=_ref)
    opcode = max(dve_ops._SUB_OPCODE_FOR_NAME.values()) + 1
    assert opcode < 0x20
    shas = {v: DveOpSpec(name=name, opcode=opcode, uops=lower(spec, ver=v),
                         rd1_en=True).sha(v) for v in ("v3", "v4")}
    op = DveOp.__new__(DveOp)
    object.__setattr__(op, "name", name)
    object.__setattr__(op, "spec", spec)
    object.__setattr__(op, "subdim", False)
    object.__setattr__(op, "uops_sha", shas)
    object.__setattr__(op, "perf_en", {})
    dve_ops.OPS.append(op)
    dve_ops.CUSTOM_DVE_SPECS[name] = spec
    dve_ops._SUB_OPCODE_FOR_NAME[name] = opcode
    return op


SOFTSIGN_OP = _register_softsign_op()


def _register_recip_op():
    """r = NR1(1 + |Src0|) in one DVE op (inline maxx-abs, seed, one
    Newton pass). Used with a following tensor_tensor mult for groups
    whose |m| would otherwise queue on ACT."""
    name = "ABS1_RECIP_ANT"
    for existing in dve_ops.OPS:
        if existing.name == name:
            return existing

    def _ref(in0, in1, c0, c1, c2):
        d = np.abs(in0.astype(np.float32)) + 1.0
        not_d = (~d.view(np.int32)).view(np.float32)
        y0 = not_d * c0
        return y0 * (c1 - d * y0)

    from concourse.dve_spec import Zero, maxx
    absm = maxx(Src0, Bin(DAlu.SUBTRACT, Zero, Src0))
    d = Bin(DAlu.ADD, absm, One)
    not_d = Bin(DAlu.BITWISE_NOT, d, d)
    y0 = not_d * C0
    spec = Spec(body=y0 * (C1 - d * y0), reference=_ref)
    opcode = max(dve_ops._SUB_OPCODE_FOR_NAME.values()) + 1
    assert opcode < 0x20
    shas = {v: DveOpSpec(name=name, opcode=opcode, uops=lower(spec, ver=v),
                         rd1_en=False).sha(v) for v in ("v3", "v4")}
    op = DveOp.__new__(DveOp)
    object.__setattr__(op, "name", name)
    object.__setattr__(op, "spec", spec)
    object.__setattr__(op, "subdim", False)
    object.__setattr__(op, "uops_sha", shas)
    object.__setattr__(op, "perf_en", {})
    dve_ops.OPS.append(op)
    dve_ops.CUSTOM_DVE_SPECS[name] = spec
    dve_ops._SUB_OPCODE_FOR_NAME[name] = opcode
    return op


RECIP_OP = _register_recip_op()


def split_drain_waits(nc):
    """Walrus (2026-05) refuses instructions with too many sync waits
    ("Too many sync wait commands", setupSyncWait): InstDrain takes at most
    1, other instructions at most 2. Move extras onto preceding single-wait
    NoOps on the same engine."""
    n_new = 0
    for fn in nc.m.functions:
        for blk in fn.blocks:
            out, changed = [], False
            for inst in blk.instructions:
                si = inst.sync_info
                cap = 1 if isinstance(inst, mybir.InstDrain) else 2
                if si is not None and len(si.on_wait) > cap:
                    waits = list(si.on_wait)
                    for w in waits[:-cap]:
                        n_new += 1
                        nop = mybir.InstNoOp(
                            name=f"waitsplit-{n_new}", ins=[], outs=[])
                        nop.engine = inst.engine
                        nop.sync_info = mybir.SyncInfo(
                            on_update=[], on_wait=[w])
                        out.append(nop)
                    si.on_wait = waits[-cap:]
                    changed = True
                out.append(inst)
            if changed:
                blk.instructions = out
    return n_new


def _softsign_group(nc, pool, msgs_ap, y_ap, fd, mask_ap, abs_on_act=True):
    """y = msgs / (1 + |msgs|); msgs in PSUM fp32, y -> SBUF bf16/fp8.
    |m| on ACT (Abs shares the sigmoid table -> no reloads) or DVE
    (bitwise_and sign-clear; load balancing), then one fused custom DVE
    op for m/(1+|m|)."""
    c = RECIP_APPROX_FAST_CONSTS
    if abs_on_act:
        a_ = pool.tile([128, fd], F32, tag="ss_a")
        nc.scalar.activation(a_[:], msgs_ap, AF.Abs)
        nc.vector._custom_dve(SOFTSIGN_OP, out=y_ap, in0=a_[:], in1=msgs_ap,
                              s0=c["s0"], s1=c["s1"], imm2=c["imm2"])
    else:
        r_ = pool.tile([128, fd], F32, tag="ss_r")
        nc.vector._custom_dve(RECIP_OP, out=r_[:], in0=msgs_ap,
                              s0=c["s0"], s1=c["s1"], imm2=c["imm2"])
        nc.vector.tensor_tensor(y_ap, msgs_ap, r_[:], ALU.mult)


def build_program(CH: int, nonzero_bm2: bool, nonzero_bf2: bool) -> bass.Bass:
    """SPMD per-core program. CH = col-chunk node count (<=128)."""
    NCH = -(-NPAD // CH)                 # col chunks
    NCH_P = -(-NCH // GROUP) * GROUP     # padded per-window tile count
    NGW = NCH_P // GROUP                 # groups per window
    NGRP = NTC * NGW

    nc = bacc.Bacc("TRN2", debug=False, num_devices=NCORES)

    # packed one-hots per group: [PR_0|PC_0|..|PR_3|PC_3 | P_0..P_3]
    OHW = GROUP * 3 * 128
    featn_t = nc.dram_tensor("featn_t", [FD, NPAD], BF16, kind="ExternalInput")
    ownfeat_t = nc.dram_tensor("ownfeat_t", [FD, RANGE], BF16,
                               kind="ExternalInput")
    owntime_t = nc.dram_tensor("owntime_t", [FD, RANGE], BF16,
                               kind="ExternalInput")
    oh_pack = nc.dram_tensor("oh_pack", [NGRP, 128, 3 * GROUP, 128],
                             ONEHOT_DT, kind="ExternalInput")
    wm1 = nc.dram_tensor("wm1", [128, 2 * FD], BF16, kind="ExternalInput")
    wm2 = nc.dram_tensor("wm2", [FD, FD], BF16, kind="ExternalInput")
    wf1 = nc.dram_tensor("wf1", [3 * FD, FD], BF16, kind="ExternalInput")
    wf2 = nc.dram_tensor("wf2", [FD, FD], BF16, kind="ExternalInput")
    bm1d = nc.dram_tensor("bm1", [FD], F32, kind="ExternalInput")
    bf1d = nc.dram_tensor("bf1", [FD], F32, kind="ExternalInput")
    if nonzero_bm2:
        bm2d = nc.dram_tensor("bm2", [FD], BF16, kind="ExternalInput")
    if nonzero_bf2:
        bf2d = nc.dram_tensor("bf2", [FD], BF16, kind="ExternalInput")
    outd = nc.dram_tensor("out", [RANGE, FD], F32, kind="ExternalOutput")


    with tile.TileContext(nc) as tc:
        with (
            tc.tile_pool(name="const", bufs=1) as cst,
            tc.tile_pool(name="oh", bufs=12) as ohp,
            tc.tile_pool(name="hp", bufs=8) as hp,
            tc.tile_pool(name="grp", bufs=6) as grp,
            tc.tile_pool(name="abp", bufs=3) as abp,
            tc.tile_pool(name="ntp", bufs=2) as ntp,
            tc.tile_pool(name="ps_s", bufs=3, space="PSUM") as ps_s,
            tc.tile_pool(name="ps_m", bufs=3, space="PSUM") as ps_m,
            tc.tile_pool(name="ps_agg", bufs=1, space="PSUM") as ps_agg,
            tc.tile_pool(name="ps_misc", bufs=1, space="PSUM") as ps_misc,
        ):
            # ---- constants ----
            wm1_sb = cst.tile([128, 2 * FD], BF16)
            nc.sync.dma_start(out=wm1_sb[:], in_=wm1[:])
            wm2_sb = cst.tile([128, FD], BF16)
            nc.sync.dma_start(out=wm2_sb[:], in_=wm2[:])
            wf1_sb = cst.tile([128, 3 * FD], BF16)
            for c3 in range(3):
                nc.sync.dma_start(
                    out=wf1_sb[:, c3 * FD:(c3 + 1) * FD],
                    in_=wf1[c3 * FD:(c3 + 1) * FD, :],
                )
            wf2_sb = cst.tile([128, FD], BF16)
            nc.sync.dma_start(out=wf2_sb[:], in_=wf2[:])
            bm1_sb = cst.tile([128, 1], F32)
            nc.sync.dma_start(out=bm1_sb[:], in_=bm1d[:, None])
            bf1_sb = cst.tile([128, 1], F32)
            nc.sync.dma_start(out=bf1_sb[:], in_=bf1d[:, None])
            iden = cst.tile([128, 128], BF16)
            make_identity(nc, iden[:])
            if nonzero_bm2 or nonzero_bf2:
                ones_sb = cst.tile([1, 128], BF16)
                nc.gpsimd.memset(ones_sb[:], 1.0)
            if nonzero_bm2:
                bm2_sb = cst.tile([1, 128], BF16)
                nc.sync.dma_start(out=bm2_sb[:], in_=bm2d[None, :])
            if nonzero_bf2:
                bf2_sb = cst.tile([1, 128], BF16)
                nc.sync.dma_start(out=bf2_sb[:], in_=bf2d[None, :])

            # ---- phase B: chunk table built straight from X^T in SBUF.
            # B chunk c = X[c*CH : c*CH+128] @ Wm1[F:]; the lhsT is just an
            # arbitrary 128-column slice of XT_sb (pad columns zeroed), so
            # no DRAM roundtrip is needed at all.
            AB_sb = cst.tile([128, (NTC + NCH) * 128], FP8)
            XT_sb = cst.tile([128, NPAD + 128], BF16)
            nc.gpsimd.memset(XT_sb[:, NPAD:], 0)
            for xq in range(4):
                x0, x1 = xq * NPAD // 4, (xq + 1) * NPAD // 4
                nc.sync.dma_start(out=XT_sb[:, x0:x1], in_=featn_t[:, x0:x1])
            for c in range(NCH):
                b_ps = ps_m.tile([128, FD], F32, tag="m")
                nc.tensor.matmul(b_ps[:],
                                 lhsT=XT_sb[:, c * CH:c * CH + 128],
                                 rhs=wm1_sb[:, FD:],
                                 start=True, stop=True)
                if c % 3 != 2:
                    nc.vector.tensor_copy(
                        AB_sb[:, (NTC + c) * 128:(NTC + c + 1) * 128], b_ps[:])
                else:
                    nc.scalar.copy(
                        AB_sb[:, (NTC + c) * 128:(NTC + c + 1) * 128], b_ps[:])

            # ---- own-range A (block w<NTC of AB_sb) ----
            gT1 = cst.tile([128, NTC * 128], BF16)
            gT3 = cst.tile([128, NTC * 128], BF16)
            xo_sb = cst.tile([128, RANGE], BF16)
            nc.scalar.dma_start(out=xo_sb[:], in_=ownfeat_t[:])
            to_sb = cst.tile([128, RANGE], BF16)
            nc.scalar.dma_start(out=to_sb[:], in_=owntime_t[:])
            nc.scalar.activation(gT1[:], xo_sb[:], AF.Sigmoid)
            nc.scalar.activation(gT3[:], to_sb[:], AF.Sigmoid)
            for w in range(NTC):
                a_ps = ps_m.tile([128, FD], F32, tag="m")
                nc.tensor.matmul(a_ps[:],
                                 lhsT=xo_sb[:, w * 128:(w + 1) * 128],
                                 rhs=wm1_sb[:, :FD],
                                 start=True, stop=True)
                nc.vector.tensor_copy(AB_sb[:, w * 128:(w + 1) * 128], a_ps[:])


            # ---- edge phase (w-major: one window's aggT at a time) ----
            for w in range(NTC):
                agg_ps = ps_agg.tile([128, 128], F32, tag="agg")
                for gw in range(NGW):
                    g = w * NGW + gw
                    # real (non-padding) tiles in this group
                    tw = min(GROUP, NCH - gw * GROUP)
                    if tw % 2:
                        tw = min(GROUP, tw + 1)  # keep DR pairing even
                    s_ps = ps_s.tile([128, GROUP * 128], F32, tag="s")
                    # one DMA for all of the group's one-hots
                    oh_t = ohp.tile([128, 3 * GROUP, 128], ONEHOT_DT, tag="oh")
                    nc.sync.dma_start(out=oh_t[:], in_=oh_pack[g])
                    abase = AB_sb[:, w * 128:(w + 1) * 128]
                    for k in range(tw):
                        c = gw * GROUP + k
                        ks = slice(k * 128, (k + 1) * 128)
                        cc = min(c, NCH - 1)
                        # fused A+B expand: lhsT = [A_w | B_cc] as 2 k-subtiles
                        lhsT = bass.AP(
                            tensor=abase.tensor, offset=abase.offset,
                            ap=[list(abase.ap[0]),
                                [(NTC + cc - w) * 128, 2], [1, 128]])
                        nc.tensor.matmul(
                            s_ps[:, ks], lhsT=lhsT,
                            rhs=oh_t[:, 2 * k:2 * k + 2, :],
                            start=True, stop=True,
                            perf_mode=mybir.MatmulPerfMode.DoubleRow,
                        )
                    # h = sigmoid(S + bm1) for the group's real tiles
                    h_t = hp.tile([128, GROUP * 128], BF16, tag="h")
                    nc.scalar.activation(h_t[:, :tw * 128],
                                         s_ps[:, :tw * 128], AF.Sigmoid,
                                         bias=bm1_sb[:])
                    # msgs = h @ Wm2 (+ bm2)
                    m_ps = ps_m.tile([128, GROUP * 128], F32, tag="m")
                    for k in range(tw):
                        ks = slice(k * 128, (k + 1) * 128)
                        if nonzero_bm2:
                            nc.tensor.matmul(
                                m_ps[:, ks], lhsT=ones_sb[:], rhs=bm2_sb[:],
                                start=True, stop=False)
                        nc.tensor.matmul(
                            m_ps[:, ks], lhsT=h_t[:, ks], rhs=wm2_sb[:],
                            start=not nonzero_bm2, stop=True,
                        )
                    # y = softsign(msgs), written as fp8 for the DR scatter
                    y_t = hp.tile([128, GROUP, 128], FP8, tag="y")
                    _softsign_group(nc, grp,
                                    m_ps[:, :tw * 128],
                                    y_t[:, :tw, :].rearrange(
                                        "p j e -> p (j e)"),
                                    tw * 128, None,
                                    abs_on_act=(g % 5 >= 2))
                    # scatter: aggT += y^T-expand, two tiles per DR matmul
                    pb = 2 * GROUP
                    for kk in range(tw // 2):
                        nc.tensor.matmul(
                            agg_ps[:], lhsT=y_t[:, 2 * kk:2 * kk + 2, :],
                            rhs=oh_t[:, pb + 2 * kk:pb + 2 * kk + 2, :],
                            start=(gw == 0 and kk == 0),
                            stop=(gw == NGW - 1 and kk == tw // 2 - 1),
                            perf_mode=mybir.MatmulPerfMode.DoubleRow,
                        )

                # ---- node MLP for window w ----
                ws = slice(w * 128, (w + 1) * 128)
                gt2 = ntp.tile([128, 128], BF16, tag="gt2")
                nc.scalar.activation(gt2[:], agg_ps[:], AF.Sigmoid)
                g2_ps = ps_misc.tile([128, 128], F32, tag="misc")
                nc.tensor.matmul(g2_ps[:], lhsT=wf1_sb[:, :FD],
                                 rhs=gT1[:, ws], start=True, stop=False)
                nc.tensor.matmul(g2_ps[:], lhsT=wf1_sb[:, FD:2 * FD],
                                 rhs=gt2[:], start=False, stop=False)
                nc.tensor.matmul(g2_ps[:], lhsT=wf1_sb[:, 2 * FD:],
                                 rhs=gT3[:, ws], start=False, stop=True)
                g2_sb = ntp.tile([128, 128], BF16, tag="g2sb")
                nc.scalar.activation(g2_sb[:], g2_ps[:], AF.Sigmoid,
                                     bias=bf1_sb[:])
                o_ps = ps_misc.tile([128, 128], F32, tag="misc")
                if nonzero_bf2:
                    nc.tensor.matmul(o_ps[:], lhsT=ones_sb[:], rhs=bf2_sb[:],
                                     start=True, stop=False)
                nc.tensor.matmul(o_ps[:], lhsT=g2_sb[:], rhs=wf2_sb[:],
                                 start=not nonzero_bf2, stop=True)
                # softsign in fp32 on DVE (output precision matters here)
                oc = ntp.tile([128, 128], F32, tag="oc")
                nc.vector.tensor_copy(oc[:], o_ps[:])
                oa = ntp.tile([128, 128], F32, tag="oa")
                nc.vector.scalar_tensor_tensor(oa[:], oc[:], -1.0, oc[:],
                                               ALU.mult, ALU.max)
                nc.vector.tensor_scalar_add(oa[:], oa[:], 1.0)
                orr = ntp.tile([128, 128], F32, tag="orr")
                nc.vector.reciprocal_approx_fast(orr[:], oa[:])
                oy = ntp.tile([128, 128], F32, tag="oy")
                nc.vector.tensor_tensor(oy[:], oc[:], orr[:], ALU.mult)
                nc.sync.dma_start(out=outd[ws, :], in_=oy[:])

    nc.compile()
    split_drain_waits(nc)
    return nc


def choose_chunk(rs, cs):
    """Largest CH in {128,120,...,64} with every (window, chunk) bucket
    <= 128 edges, checked over the actual data (global windows cover all
    cores at once)."""
    w_glob = rs // 128
    for CH in (128, 124, 120, 116, 112, 110, 108, 106, 104,
               100, 96, 88, 80, 72, 64):
        nch = -(-NPAD // CH)
        bid = w_glob * nch + cs // CH
        if np.bincount(bid).max() <= 128:
            return CH
    raise ValueError("no feasible col-chunk size; graph too skewed")


def prepare_inputs(features, rows, cols, time_embedding,
                   Wm1, bm1, Wm2, bm2, Wf1, bf1, Wf2, bf2):
    features = np.asarray(features, np.float32)
    time_embedding = np.asarray(time_embedding, np.float32)
    rows = np.asarray(rows).astype(np.int64)
    cols = np.asarray(cols).astype(np.int64)
    Wm1 = np.asarray(Wm1, np.float32)
    Wm2 = np.asarray(Wm2, np.float32)
    Wf1 = np.asarray(Wf1, np.float32)
    Wf2 = np.asarray(Wf2, np.float32)
    bm1 = np.asarray(bm1, np.float32).reshape(FD)
    bm2 = np.asarray(bm2, np.float32).reshape(FD)
    bf1 = np.asarray(bf1, np.float32).reshape(FD)
    bf2 = np.asarray(bf2, np.float32).reshape(FD)

    CH = choose_chunk(rows, cols)
    NCH = -(-NPAD // CH)
    NCH_P = -(-NCH // GROUP) * GROUP
    NGW = NCH_P // GROUP
    TT_P = NTC * NCH_P
    NGRP = NTC * NGW

    feat_pad = np.zeros((NPAD, FD), np.float32)
    feat_pad[:N] = features
    time_pad = np.zeros((NPAD, FD), np.float32)
    time_pad[:N] = time_embedding
    featbf = feat_pad.astype(NPBF16)
    timebf = time_pad.astype(NPBF16)
    wm1cat = np.concatenate([Wm1[:FD], Wm1[FD:]], axis=1).astype(NPBF16)

    nonzero_bm2 = bool(np.any(bm2))
    nonzero_bf2 = bool(np.any(bf2))
    common = {
        "featn_t": np.ascontiguousarray(featbf.T),
        "wm1": wm1cat, "wm2": Wm2.astype(NPBF16),
        "wf1": Wf1.astype(NPBF16), "wf2": Wf2.astype(NPBF16),
        "bm1": bm1, "bf1": bf1,
    }
    if nonzero_bm2:
        common["bm2"] = bm2.astype(NPBF16)
    if nonzero_bf2:
        common["bf2"] = bf2.astype(NPBF16)

    in_maps = []
    for core in range(NCORES):
        base = core * RANGE
        sel = (rows >= base) & (rows < base + RANGE)
        r_c = rows[sel]
        c_c = cols[sel]
        w_loc = (r_c - base) // 128
        cch = c_c // CH
        tid = w_loc * NCH_P + cch        # window-major tile order
        order = np.argsort(tid, kind="stable")
        r_s, c_s, t_s = r_c[order], c_c[order], tid[order]
        # slot within tile
        slot = np.zeros(len(t_s), np.int64)
        if len(t_s):
            newt = np.r_[True, t_s[1:] != t_s[:-1]]
            starts = np.nonzero(newt)[0]
            slot = np.arange(len(t_s)) - np.repeat(starts, np.diff(
                np.r_[starts, len(t_s)]))
        assert slot.max(initial=0) < 128, "bucket overflow"
        epos = t_s * 128 + slot          # edge position in tile grid
        rrel = (r_c[order] - base) % 128
        crel = c_s - (t_s % NCH_P) * CH

        P = np.zeros((TT_P * 128, 128), np.float32)
        P[epos, rrel] = 1.0
        PC = np.zeros((TT_P * 128, 128), np.float32)
        PC[epos, crel] = 1.0

        # packed group layout: [PR_0|PC_0|..|PR_3|PC_3 | P_0..P_3]
        P4 = P.reshape(NGRP, GROUP, 128, 128)     # [g, k, e, n]
        PC4 = PC.reshape(NGRP, GROUP, 128, 128)
        expand = np.stack(
            [P4.transpose(0, 1, 3, 2), PC4.transpose(0, 1, 3, 2)], axis=2
        )                                          # [g, k, {pr,pc}, n, e]
        expand = expand.transpose(0, 3, 1, 2, 4)   # [g, n, k, 2, e]
        scatter = P4.transpose(0, 2, 1, 3)         # [g, e, k, n]
        ohp = np.concatenate(
            [expand.reshape(NGRP, 128, GROUP * 2, 128),
             scatter.reshape(NGRP, 128, GROUP, 128)], axis=2)

        m = dict(common)
        m["oh_pack"] = np.ascontiguousarray(ohp.astype(ONEHOT_NP))
        m["ownfeat_t"] = np.ascontiguousarray(featbf[base:base + RANGE].T)
        m["owntime_t"] = np.ascontiguousarray(timebf[base:base + RANGE].T)
        in_maps.append(m)

    return CH, nonzero_bm2, nonzero_bf2, in_maps


def kernel(features, rows, cols, time_embedding,
           Wm1, bm1, Wm2, bm2, Wf1, bf1, Wf2, bf2) -> np.ndarray:
    CH, nz_bm2, nz_bf2, in_maps = prepare_inputs(
        features, rows, cols, time_embedding,
        Wm1, bm1, Wm2, bm2, Wf1, bf1, Wf2, bf2,
    )
    nc = build_program(CH, nz_bm2, nz_bf2)
    res = run_bass_kernel_spmd(nc, in_maps, list(range(NCORES)))
    out = np.concatenate(
        [res.results[c]["out"] for c in range(NCORES)], axis=0
    )[:N]
    return np.ascontiguousarray(out.astype(np.float32))

